# revision 37
# baseline (speedup 1.0000x reference)
"""Trainium2 Bass kernel: fc1+relu -> LSTM(H=32, T=200) -> fc2 on last hidden.

Data parallel over 8 NeuronCores: batch 4096 -> 512 per core, 4 btiles x 128.

v3 (chain-latency optimized vs baseline):
  - Same macro-structure as baseline: two streams (btiles {0,1} / {2,3}),
    K-augmented single gate matmul per btile (K=53 stationary = [h^T|x1aug^T]),
    all-tanh gates (sigmoid folded), C=2c / H2=2h scaling.
  - PE p-state fillers: dummy matmuls (wcomb x Q-slot -> dump bank) keep the
    tensor engine continuously busy so real transposes/matmuls run at full
    clock instead of the cold 0.65GHz (measured 284ns -> 106ns per 128-col).
  - Cell state C lives in SBUF fp16 inside a per-step SCell tile
    [C(64) | A(256)] so the gates tanh, u/v/C' stt ops and cell tanh all hit
    cheap SBUF access paths (PSUM access costs ~2x on DVE).
  - fp16 everywhere on-chip (2-byte DVE fast paths + more mantissa than bf16).
  - fc1 for chunk ci+1 is emitted inside chunk ci's t-loop (sprinkled), so
    phase A overlaps the recurrence instead of serializing ~30us up front.
  - One output DMA (rearranged AP) instead of 4.
"""

import os
import sys
import numpy as np
from contextlib import ExitStack

sys.path.insert(0, "/opt/trn_rl_repo")
sys.path.insert(0, "/opt/pypackages")

import concourse.bass as bass
import concourse.bacc as bacc
import concourse.tile as tile
import concourse.mybir as mybir
from concourse import bass_utils
from concourse.masks import make_identity

F32 = mybir.dt.float32
F16 = mybir.dt.float16 if os.environ.get("K_DT", "f16") == "f16" else mybir.dt.bfloat16
AF = mybir.ActivationFunctionType
ALU = mybir.AluOpType

H = 32
B = 4096
T = 200
CIN = 5
C6 = 6
NCORES = 8
BL = B // NCORES  # 512
NBT = BL // 128  # 4
TCH = 20  # timesteps per chunk
NCH = T // TCH  # 10
QW = 64  # per-(t,btile) block width in Q: [H2(32) | h1aug(21) | pad(11)]
QROW = NBT * QW  # 256 per timestep

FILL_N = int(os.environ.get("K_FILL_N", "0"))
FILL_F = int(os.environ.get("K_FILL_F", "256"))
WARM_N = int(os.environ.get("K_WARM", "0"))
V_POOL = int(os.environ.get("K_V_POOL", "0"))
C_PSUM = int(os.environ.get("K_C_PSUM", "0"))
UV = int(os.environ.get("K_UV", "1"))
EMIT2 = int(os.environ.get("K_EMIT", "0"))
QL = int(os.environ.get("K_QL", "0"))
# Q slot layout per timestep (QROW=256 cols):
#  QL=0: [H2(32)|x1aug(21)|pad(11)] x 4 btiles (64-stride slots)
#  QL=1: [H2 x4 (128) | x1aug-pad32 x4 (128)] -> contiguous H2 stores

# gate blocks: 0=f, 1=i, 2=g, 3=o ; torch gate order is i,f,g,o
_TORCH_BASE = {0: 32, 1: 0, 2: 64, 3: 96}
# column-block order within each btile's 128 gate columns
_ORD = [2, 0, 1, 3] if UV else [0, 1, 2, 3]  # UV: [g f i o], else [f i g o]


def _perm_scale():
    perm = np.zeros(4 * H, dtype=np.int64)
    srow = np.zeros(4 * H, dtype=np.float32)
    for j in range(4 * H):
        blk, idx = _ORD[j // H], j % H
        perm[j] = _TORCH_BASE[blk] + idx
        srow[j] = 1.0 if blk == 2 else 0.5
    return perm, srow


def prep_consts(fc1_w, fc1_b, w_ih, w_hh, b_ih, b_hh, fc2_w, fc2_b):
    perm, srow = _perm_scale()
    # wcomb [53,128]: rows 0:32 h-side (x0.5 for the H2=2h scaling), rows
    # 32:52 fc1-side, row 52 bias. Columns are [f|i|g|o]x32 with the tanh
    # half-angle scaling folded in (srow).
    wcomb = np.zeros((53, 128), np.float32)
    wcomb[0:32] = 0.5 * (srow[:, None] * w_hh[perm]).T
    wcomb[32:52] = (srow[:, None] * w_ih[perm]).T
    wcomb[52] = srow * (b_ih + b_hh)[perm]
    # w1bd [120, 21*TCH]: block-diagonal fc1 (+bias via c=5 row, ones col 20)
    w1bd = np.zeros((C6 * TCH, 21 * TCH), np.float32)
    for w in range(TCH):
        for c in range(CIN):
            w1bd[C6 * w + c, 21 * w : 21 * w + 20] = fc1_w[:, c]
        w1bd[C6 * w + CIN, 21 * w : 21 * w + 20] = fc1_b
        w1bd[C6 * w + CIN, 21 * w + 20] = 1.0
    fc2w_rep = np.ascontiguousarray(0.5 * fc2_w.T)  # [32,2]
    f16 = np.float16
    return dict(
        wcomb=wcomb.astype(f16), w1bd=w1bd.astype(f16), fc2w_rep=fc2w_rep.astype(f16)
    )


def emit(tc, outs, ins):
    nc = tc.nc
    ctx = ExitStack()
    xd = ins["x"]  # [512, 1000]
    out_d = outs["out"]  # [512, 2]

    consts = ctx.enter_context(tc.tile_pool(name="consts", bufs=1))
    ident = consts.tile([128, 128], F16, tag="ident")
    make_identity(nc, ident[:])
    wcomb = consts.tile([53, 128], F16, tag="wcomb")
    nc.sync.dma_start(wcomb[:], ins["wcomb"][:, :])
    w1bd = consts.tile([C6 * TCH, 21 * TCH], F16, tag="w1bd")
    nc.sync.dma_start(w1bd[:], ins["w1bd"][:, :])
    fc2w = consts.tile([32, 2], F16, tag="fc2w")
    nc.sync.dma_start(fc2w[:], ins["fc2w_rep"][:, :])
    wfill = consts.tile([53, 512], F16, tag="wfill")
    nc.vector.memset(wfill[:], 0.25)

    # ---------------- pools ----------------
    xpool = ctx.enter_context(tc.tile_pool(name="x6", bufs=1))
    xsb_pool = ctx.enter_context(tc.tile_pool(name="xsb", bufs=2))
    psum = ctx.enter_context(tc.tile_pool(name="ps", bufs=1, space="PSUM"))
    xt_pool = ctx.enter_context(tc.tile_pool(name="xt", bufs=2))
    q_pool = ctx.enter_context(tc.tile_pool(name="q", bufs=5))
    sc_pool = ctx.enter_context(tc.tile_pool(name="sc", bufs=2))
    work = ctx.enter_context(tc.tile_pool(name="wk", bufs=2))

    dump = None
    if FILL_N > 0 or WARM_N > 0:
        dump = psum.tile([128, 512], F32, tag="dump", bufs=1)
    cst = None
    if C_PSUM and not UV:
        # cell state in PSUM (ACT reads PSUM faster than SBUF): uses the bank
        # freed by the (disabled) fillers
        cst = psum.tile([128, 128], F32, tag="cst", bufs=1)
        nc.vector.memset(cst[:], 0.0)

    # ---------------- x load: host pre-padded to 6 fp16 channels ----------
    x6 = [
        xpool.tile([128, C6 * T], F16, tag=f"x6_{k}", name=f"x6_{k}")
        for k in range(NBT)
    ]
    for k in range(NBT):
        nc.sync.dma_start(x6[k][:], xd[128 * k : 128 * (k + 1), :])

    qc = [
        q_pool.tile([128, TCH * QROW], F16, tag="qc", name=f"qc_{ci}")
        for ci in range(NCH)
    ]
    qf = q_pool.tile([128, QROW], F16, tag="qf", bufs=1)
    nc.vector.memset(qf[:], 0.0)
    # zero the H2 slots of chunk 0, slot w=0 (h_{-1} = 0)
    nc.vector.memset(qc[0][:, 0:QROW], 0.0)

    # ---------------- fc1 emission units (per chunk) ----------------
    # each chunk's fc1 = [4x x-transpose, 1x copy, 4x matmul, 4x relu]
    fc1_state = {}

    def fc1_unit(ci, unit):
        """Emit one fc1 emission unit for chunk ci. Units 0-3: transposes,
        4: psum->sbuf copy, 5-12: alternating matmul/relu per btile."""
        if unit == 0:
            fc1_state[ci] = {}
            fc1_state[ci]["xtp"] = psum.tile(
                [C6 * TCH, 512], F16, tag="xtp", bufs=1, name=f"xtp_{ci}"
            )
        st = fc1_state[ci]
        if unit < 4:
            k = unit
            nc.tensor.transpose(
                st["xtp"][:, 128 * k : 128 * (k + 1)],
                x6[k][:, C6 * TCH * ci : C6 * TCH * (ci + 1)],
                ident[:],
            )
        elif unit == 4:
            st["xt"] = xt_pool.tile([C6 * TCH, 512], F16, tag="xt", name=f"xt_{ci}")
            nc.vector.tensor_copy(st["xt"][:], st["xtp"][:])
        elif unit % 2 == 1:  # 5,7,9,11 -> matmul k=0..3
            k = (unit - 5) // 2
            fps = psum.tile(
                [128, 21 * TCH], F32, tag="fc1", bufs=2, name=f"fps_{ci}_{k}"
            )
            st["fps"] = fps
            nc.tensor.matmul(
                fps[:],
                st["xt"][:, 128 * k : 128 * (k + 1)],
                w1bd[:],
                start=True,
                stop=True,
                tile_position=(0, 0),
            )
        else:  # 6,8,10,12 -> relu k=0..3
            k = (unit - 6) // 2
            qv = qc[ci][:].rearrange("p (w b) -> p w b", b=QROW)
            x1off = (128 + 32 * k) if QL else (QW * k + 32)
            nc.scalar.activation(
                qv[:, :, x1off : x1off + 21],
                st["fps"][:].rearrange("p (w m) -> p w m", m=21),
                AF.Relu,
            )
            if k == 3:
                del fc1_state[ci]

    # chunk 0 fully up front
    for u in range(13):
        fc1_unit(0, u)

    # PE warmup fillers (no deps beyond consts; scheduler will front-load)
    for i in range(WARM_N):
        nc.tensor.matmul(
            dump[:, 0:FILL_F],
            wcomb[:],
            wfill[:, 0:FILL_F],
            start=True,
            stop=True,
            tile_position=(0, 0),
        )

    # ---------------- recurrence ----------------
    # SCell per stream:
    #  UV=0: [C (64: j0|j1) | A (256: [f i g o]x32 per btile x2)]
    #  UV=1: [128, 640], per btile j (320-col span):
    #        C_j@+0, TCT_j@+32, gates [g f i o]@+160:320
    #        -> UV-stt in1 [C_j0, g_j0, C_j1, g_j1] is uniform stride-160
    SCW = 640 if UV else 320
    scell = []
    for s in range(2):
        sc = sc_pool.tile([128, SCW], F16, tag=f"sc{s}", name=f"sc{s}_init")
        if UV:
            cv = sc[:].rearrange("p (k r) -> p k r", r=320)
            nc.vector.memset(cv[:, :, 128:160], 0.0)  # zero the C slots
        else:
            nc.vector.memset(sc[:, 0:64], 0.0)
        scell.append(sc)

    # sprinkle schedule for chunk ci+1's 13 fc1 units inside chunk ci:
    # unit u emitted at (w = u+2, after stream 1)
    for t in range(T):
        ci, w = t // TCH, t % TCH
        if t + 1 < T:
            cin, wn = (t + 1) // TCH, (t + 1) % TCH
            qdst = qc[cin][:, QROW * wn : QROW * (wn + 1)]
        else:
            qdst = qf[:]
        qv4 = qdst.rearrange("p (k s) -> p k s", s=QW)

        def front(s):
            """T + L-copy + gate matmuls for stream s; returns (L, gt)."""
            TPH = 64 if QL else 53
            tp = psum.tile([TPH, 256], F16, tag=f"tp{s}", bufs=1, name=f"tp{s}_{t}")
            qslot = qc[ci][:, QROW * w : QROW * (w + 1)]
            qsv = qslot.rearrange("p (a k c) -> p a k c", a=2, c=32)
            for j in range(2):
                k = 2 * s + j
                tin = (
                    qsv[:, :, k, :]
                    if QL
                    else qc[ci][:, QROW * w + QW * k : QROW * w + QW * k + 53]
                )
                nc.tensor.transpose(tp[:, 128 * j : 128 * (j + 1)], tin, ident[:])
            L = work.tile([TPH, 256], F16, tag=f"L{s}", name=f"L{s}_{t}")
            nc.vector.tensor_copy(L[:], tp[:])
            gt = psum.tile([128, 256], F32, tag=f"g{s}", bufs=1, name=f"g{s}_{t}")
            for j in range(2):
                nc.tensor.matmul(
                    gt[:, 128 * j : 128 * (j + 1)],
                    L[0:53, 128 * j : 128 * (j + 1)],
                    wcomb[:],
                    start=True,
                    stop=True,
                    tile_position=(0, 0),
                )
            return L, gt

        def gates_uv(s, gt):
            sc = scell[s]
            scv = sc[:].rearrange("p (k r) -> p k r", r=320)
            nc.scalar.activation(scv[:, :, 160:288], gt[:], AF.Tanh)

        def cell_uv(s, L):
            sc = scell[s]
            scv = sc[:].rearrange("p (k r) -> p k r", r=320)
            uvt = work.tile([128, 128], F16, tag=f"uv{s}", name=f"uv{s}_{t}")
            # in1 = [C_j, g_j] adjacent (2-run pattern)
            nc.vector.scalar_tensor_tensor(
                uvt[:], scv[:, :, 192:256], 1.0, scv[:, :, 128:192], ALU.add, ALU.mult
            )
            scn = sc_pool.tile([128, SCW], F16, tag=f"sc{s}", name=f"sc{s}_{t}")
            scnv = scn[:].rearrange("p (k r) -> p k r", r=320)
            uv2 = uvt[:].rearrange("p (k r) -> p k r", r=64)
            nc.vector.scalar_tensor_tensor(
                scnv[:, :, 128:160],
                uv2[:, :, 0:32],
                0.5,
                uv2[:, :, 32:64],
                ALU.mult,
                ALU.add,
            )
            tct = work.tile([128, 64], F16, tag=f"tc{s}", name=f"tc{s}_{t}")
            nc.scalar.activation(tct[:], scnv[:, :, 128:160], AF.Tanh, scale=0.5)
            h2dst = (
                qdst[:, 64 * s : 64 * s + 64]
                if QL
                else qv4[:, 2 * s : 2 * s + 2, 0:32]
            )
            nc.vector.scalar_tensor_tensor(
                h2dst, scv[:, :, 256:288], 1.0, tct[:], ALU.add, ALU.mult
            )
            scell[s] = scn
            for fi in range(FILL_N):
                nc.tensor.matmul(
                    dump[:, 0:FILL_F],
                    wcomb[:],
                    L[0:53, 0:FILL_F],
                    start=True,
                    stop=True,
                    tile_position=(0, 0),
                )

        if UV and EMIT2:
            # interleave so stream B's L-copy precedes stream A's cell ops
            # in the DVE queue (kills the copy-vs-C' contention slot)
            L0, gt0 = front(0)
            gates_uv(0, gt0)
            L1, gt1 = front(1)
            cell_uv(0, L0)
            gates_uv(1, gt1)
            cell_uv(1, L1)
            if ci + 1 < NCH and 2 <= w < 15:
                fc1_unit(ci + 1, w - 2)
            continue
        if UV:
            for s in range(2):
                L, gt = front(s)
                gates_uv(s, gt)
                cell_uv(s, L)
            if ci + 1 < NCH and 2 <= w < 15:
                fc1_unit(ci + 1, w - 2)
            continue
        for s in range(2):
            L, gt = front(s)
            # --- gates tanh into SCell A-region
            sc = scell[s]
            if False:
                scv = sc[:].rearrange("p (k r) -> p k r", r=320)
                sc160 = sc[:].rearrange("p (k r) -> p k r", r=160)
                nc.scalar.activation(scv[:, :, 160:288], gt[:], AF.Tanh)
                # --- u|v in ONE stt: (tf+1)*C and (ti+1)*tg interleaved
                uvt = work.tile([128, 128], F16, tag=f"uv{s}", name=f"uv{s}_{t}")
                nc.vector.scalar_tensor_tensor(
                    uvt[:],
                    scv[:, :, 192:256],
                    1.0,
                    sc160[:, :, 0:32],
                    ALU.add,
                    ALU.mult,
                )
                scn = sc_pool.tile([128, SCW], F16, tag=f"sc{s}", name=f"sc{s}_{t}")
                scnv = scn[:].rearrange("p (k r) -> p k r", r=320)
                uv2 = uvt[:].rearrange("p (k r) -> p k r", r=64)
                nc.vector.scalar_tensor_tensor(
                    scnv[:, :, 0:32],
                    uv2[:, :, 0:32],
                    0.5,
                    uv2[:, :, 32:64],
                    ALU.mult,
                    ALU.add,
                )
                # --- cell tanh: C' (next sc) -> TCT slots of current sc
                nc.scalar.activation(
                    scv[:, :, 32:64], scnv[:, :, 0:32], AF.Tanh, scale=0.5
                )
                # --- H2 = (to+1)*tct
                h2dst = (
                    qdst[:, 64 * s : 64 * s + 64]
                    if QL
                    else qv4[:, 2 * s : 2 * s + 2, 0:32]
                )
                nc.vector.scalar_tensor_tensor(
                    h2dst,
                    scv[:, :, 256:288],
                    1.0,
                    scv[:, :, 32:64],
                    ALU.add,
                    ALU.mult,
                )
                scell[s] = scn
                for fi in range(FILL_N):
                    nc.tensor.matmul(
                        dump[:, 0:FILL_F],
                        wcomb[:],
                        L[0:53, 0:FILL_F],
                        start=True,
                        stop=True,
                        tile_position=(0, 0),
                    )
                continue
            av = sc[:, 64:320].rearrange("p (k g) -> p k g", k=2)
            nc.scalar.activation(sc[:, 64:320], gt[:], AF.Tanh)
            cs = cst[:, 64 * s : 64 * (s + 1)] if C_PSUM else sc[:, 0:64]
            # --- u, v into one UV tile
            uvt = work.tile([128, 128], F16, tag=f"uv{s}", name=f"uv{s}_{t}")
            nc.vector.scalar_tensor_tensor(
                uvt[:, 0:64], av[:, :, 0:32], 1.0, cs, ALU.add, ALU.mult
            )
            nc.vector.scalar_tensor_tensor(
                uvt[:, 64:128], av[:, :, 32:64], 1.0, av[:, :, 64:96], ALU.add, ALU.mult
            )
            # --- C' = 0.5u + v (in place for PSUM C, else next SCell's C slot)
            scn = sc_pool.tile([128, SCW], F16, tag=f"sc{s}", name=f"sc{s}_{t}")
            cdst = cs if C_PSUM else scn[:, 0:64]
            nc.vector.scalar_tensor_tensor(
                cdst, uvt[:, 0:64], 0.5, uvt[:, 64:128], ALU.mult, ALU.add
            )
            # --- cell tanh
            tct = work.tile([128, 64], F16, tag=f"tc{s}", name=f"tc{s}_{t}")
            nc.scalar.activation(tct[:], cdst, AF.Tanh, scale=0.5)
            # --- H2 = (to+1)*tc -> Q slots for t+1
            h2dst = (
                qdst[:, 64 * s : 64 * s + 64]
                if QL
                else qv4[:, 2 * s : 2 * s + 2, 0:32]
            )
            nc.vector.scalar_tensor_tensor(
                h2dst, av[:, :, 96:128], 1.0, tct[:], ALU.add, ALU.mult
            )
            scell[s] = scn
            # --- p-state fillers: moving = this stream's L (dep = the L copy,
            # which fires before this stream's matmuls even run). They sit in
            # the PE FIFO right after the matmuls, run with zero wait, and
            # keep the PE busy through the cell-phase idle gap so the next
            # transpose starts at a warm p-state.
            for fi in range(FILL_N):
                nc.tensor.matmul(
                    dump[:, 0:FILL_F],
                    wcomb[:],
                    L[0:53, 0:FILL_F],
                    start=True,
                    stop=True,
                    tile_position=(0, 0),
                )
        # --- sprinkle next chunk's fc1
        if ci + 1 < NCH and 2 <= w < 15:
            fc1_unit(ci + 1, w - 2)

    # ---------------- fc2 ----------------
    f2p = psum.tile([128, 8], F32, tag="xtp", bufs=1, name="f2p")
    for s in range(2):
        tpf = psum.tile([53, 256], F16, tag=f"tp{s}", bufs=1, name=f"tpf{s}")
        for j in range(2):
            k = 2 * s + j
            h2off = 32 * k if QL else QW * k
            nc.tensor.transpose(
                tpf[0:32, 128 * j : 128 * (j + 1)],
                qf[:, h2off : h2off + 32],
                ident[:],
            )
        Lf = work.tile([53, 256], F16, tag=f"L{s}", name=f"Lf{s}")
        nc.vector.tensor_copy(Lf[0:32, :], tpf[0:32, :])
        for j in range(2):
            k = 2 * s + j
            nc.tensor.matmul(
                f2p[:, 2 * k : 2 * k + 2],
                Lf[0:32, 128 * j : 128 * (j + 1)],
                fc2w[:],
                start=True,
                stop=True,
                tile_position=(0, 0),
            )
    f2s = work.tile([128, 8], F32, tag="f2s", name="f2s")
    nc.vector.tensor_copy(f2s[:], f2p[:])
    # natural-layout DMA (contiguous 32B runs); host reorders [128,(4,2)]->[512,2]
    nc.sync.dma_start(out_d[:], f2s[:])
    ctx.close()


_CACHE = {}


def _build():
    if "nc" in _CACHE:
        return _CACHE["nc"]
    nc = bacc.Bacc(
        "TRN2",
        target_bir_lowering=False,
        debug=False,
        enable_asserts=False,
        num_devices=NCORES,
    )
    ins = {
        "x": nc.dram_tensor("x", [BL, C6 * T], F16, kind="ExternalInput").ap(),
        "wcomb": nc.dram_tensor("wcomb", [53, 128], F16, kind="ExternalInput").ap(),
        "w1bd": nc.dram_tensor(
            "w1bd", [C6 * TCH, 21 * TCH], F16, kind="ExternalInput"
        ).ap(),
        "fc2w_rep": nc.dram_tensor(
            "fc2w_rep", [32, 2], F16, kind="ExternalInput"
        ).ap(),
    }
    outs = {"out": nc.dram_tensor("out", [128, 8], F32, kind="ExternalOutput").ap()}
    with tile.TileContext(nc) as tc:
        emit(tc, outs, ins)
    nc.compile()
    _CACHE["nc"] = nc
    return nc


def make_in_maps(x, fc1_w, fc1_b, w_ih, w_hh, b_ih, b_hh, fc2_w, fc2_b):
    consts = prep_consts(fc1_w, fc1_b, w_ih, w_hh, b_ih, b_hh, fc2_w, fc2_b)
    in_maps = []
    for c in range(NCORES):
        x6 = np.ones((BL, T, C6), np.float16)
        x6[:, :, 0:CIN] = x[c * BL : (c + 1) * BL].reshape(BL, T, CIN)
        in_maps.append({"x": np.ascontiguousarray(x6.reshape(BL, C6 * T)), **consts})
    return in_maps


def kernel(x, fc1_w, fc1_b, w_ih, w_hh, b_ih, b_hh, fc2_w, fc2_b, trace=False):
    x = np.asarray(x, np.float32)
    args = [
        np.asarray(a, np.float32)
        for a in (fc1_w, fc1_b, w_ih, w_hh, b_ih, b_hh, fc2_w, fc2_b)
    ]
    nc = _build()
    in_maps = make_in_maps(x, *args)
    res = bass_utils.run_bass_kernel_spmd(
        nc, in_maps, core_ids=list(range(NCORES)), trace=trace
    )
    # per-core out is [128, (btile, 2)]; reorder to [512, 2]
    out = np.concatenate(
        [
            r["out"].reshape(128, NBT, 2).transpose(1, 0, 2).reshape(BL, 2)
            for r in res.results
        ],
        axis=0,
    )
    out = out + args[7][None, :]
    if trace:
        kernel.last_results = res
    return out.astype(np.float32)
